# revision 31
# baseline (speedup 1.0000x reference)
"""GAU (gated attention unit) Trainium2 Bass kernel, v3.

Sharding: 8 cores = 4 batches x 2 E-halves.
  core c -> batch b = c//2, E-half h = c%2 (cols h*768:(h+1)*768 of E=1536).

v1 (~272us) -> v2 (~227us): kept-key compaction (half the keys are
masked out; gather survivors, padded to NK*128), LN centering folded
into the weights on host (exact), fused single pass.

v2 -> v3: engine rebalance so PSUM evacuation never stalls the PE.
- sqrt(rstd) folds: q'' = q_true * sqrt(rstd_q) per query column and
  k'' = k_true * sqrt(rstd_k) per key column. relu^2 is homogeneous of
  degree 2, so a'' = rstd_q[qt] * rstd_k[kt] * a_true. The rstd_k factor
  is exactly v's LN scale (v evac becomes a plain copy), and the rstd_q
  factor is exactly u's LN scale, which distributes over the final
  contraction (u and out evacs become plain copies). All rstd work now
  lives in the q/k affines as row broadcasts of rstd^1.5 and rstd^0.5:
    q'' = zq_raw * (rstd^1.5 * gamma_q) + beta_q * rstd^0.5
- LN stat chains: DVE bn_stats -> batched sqrt/recip finalizers; the 12
  full-T chains for query blocks 1..3 run inside the main loop where the
  DVE has slack. Rows are built with DVE 32x32 block transposes + block
  DMAs to DRAM in token order + stride-0 broadcast reads.
- relu alternates DVE/ACT; squares on DVE in fp16 (2x); k affine runs
  mostly in fp16.

All matmuls use fp16 operands with fp32 PSUM accumulation. Measured
rel err vs the fp32 reference ~9e-4.
"""

import numpy as np
from contextlib import ExitStack

import concourse.bass as bass
import concourse.tile as tile
from concourse import bacc, mybir
from concourse.bass_utils import run_bass_kernel_spmd

# Problem dims (hardcoded per the task contract)
B, T, D, S, E = 4, 2048, 768, 128, 1536
EH = E // 2          # per-core E half
P = 128
ND = D // P          # 6 d-chunks
NT = T // P          # 16 token chunks
TT = 512             # query block size
NTT = T // TT        # 4
LN_EPS = 1e-5

F32 = mybir.dt.float32
FP16 = mybir.dt.float16
AF = mybir.ActivationFunctionType
ALU = mybir.AluOpType
NPFP16 = np.float16

N_CORES = 8


def _segs(total, step):
    out = []
    o = 0
    while o < total:
        out.append((o, min(step, total - o)))
        o += step
    return out


def build_module(NK, with_bu=False, with_bv=False):
    TK = NK * P          # padded kept-key count
    nc = bacc.Bacc("TRN2", debug=False, num_devices=N_CORES, num_swdge_queues=4)

    # ---- DRAM I/O ----
    xT_d = nc.dram_tensor("xT", [D, T], FP16, kind="ExternalInput").ap()
    x_d = nc.dram_tensor("x", [T, D], FP16, kind="ExternalInput").ap()
    xkT_d = nc.dram_tensor("xkT", [D, TK], FP16, kind="ExternalInput").ap()
    xk_d = nc.dram_tensor("xk", [TK, D], FP16, kind="ExternalInput").ap()
    gq_d = nc.dram_tensor("gqT", [S, T], FP16, kind="ExternalInput").ap()
    bq_d = nc.dram_tensor("bqT", [S, T], FP16, kind="ExternalInput").ap()
    gk_d = nc.dram_tensor("gkT", [S, TK], FP16, kind="ExternalInput").ap()
    bk_d = nc.dram_tensor("bkT", [S, TK], FP16, kind="ExternalInput").ap()
    wz_d = nc.dram_tensor("Wz", [D, S], FP16, kind="ExternalInput").ap()
    wu_d = nc.dram_tensor("Wu", [D, EH], FP16, kind="ExternalInput").ap()
    wv_d = nc.dram_tensor("Wv", [D, EH], FP16, kind="ExternalInput").ap()
    wo_d = nc.dram_tensor("Wo", [EH, D], FP16, kind="ExternalInput").ap()
    out_d = nc.dram_tensor("outp", [T, D], FP16, kind="ExternalOutput").ap()
    # scratch rows for the rstd^1.5 / rstd^0.5 bounces (token order)
    s15_d = nc.dram_tensor("r15_scr", [NT, P], F32, kind="Internal").ap()
    s05_d = nc.dram_tensor("r05_scr", [NT, P], F32, kind="Internal").ap()
    s15k_d = nc.dram_tensor("r15k_scr", [NK, P], F32, kind="Internal").ap()
    s05k_d = nc.dram_tensor("r05k_scr", [NK, P], F32, kind="Internal").ap()
    if with_bu:
        bu_d = nc.dram_tensor("bu", [EH], F32, kind="ExternalInput").ap()
    if with_bv:
        bv_d = nc.dram_tensor("bv", [1, EH], FP16, kind="ExternalInput").ap()

    xT_r = xT_d.rearrange("(c p) t -> p c t", p=P)     # [128, 6, T]
    xkT_r = xkT_d.rearrange("(c p) t -> p c t", p=P)   # [128, 6, TK]
    x_r2 = x_d.rearrange("(ch p) d -> p ch d", p=P)    # [128, 16, D]
    xk_r2 = xk_d.rearrange("(ch p) d -> p ch d", p=P)  # [128, NK, D]
    wu_r = wu_d.rearrange("(c p) e -> p c e", p=P)
    wv_r = wv_d.rearrange("(c p) e -> p c e", p=P)
    wz_r = wz_d.rearrange("(c p) s -> p c s", p=P)
    wo_r = wo_d.rearrange("(c p) d -> p c d", p=P)

    with tile.TileContext(nc) as tc, ExitStack() as ctx:
        # ---------- persistent pools ----------
        persist = ctx.enter_context(tc.tile_pool(name="persist", bufs=1))
        eps_t = persist.tile([P, 1], F32)
        nc.vector.memset(eps_t, LN_EPS)
        warm = persist.tile([P, 1], F32)
        nc.scalar.activation(out=warm, in_=eps_t, func=AF.Sqrt)

        r15_col = persist.tile([P, 32], F32)   # token-major rstd^1.5, full T
        r05_col = persist.tile([P, 32], F32)
        r15k_col = persist.tile([P, 32], F32)  # token-major, kept tokens
        r05k_col = persist.tile([P, 32], F32)
        for t_ in (r15_col, r05_col, r15k_col, r05k_col):
            nc.vector.memset(t_, 0.0)
        mvs_f = persist.tile([P, NT, 2], F32)  # bn_aggr (mean, var) full T
        mvs_k = persist.tile([P, 32, 2], F32)  # bn_aggr (mean, var) kept
        r15b = persist.tile([P, T], F32)       # rstd^1.5 row, all partitions
        r05b = persist.tile([P, T], F32)
        r15kb = persist.tile([P, TK], F32)
        r05kb = persist.tile([P, TK], F32)
        bq2 = persist.tile([S, T], FP16)       # beta_q * rstd^0.5
        bk2 = persist.tile([S, TK], FP16)
        kT = persist.tile([S, TK], FP16)
        v_t = persist.tile([P, NK, EH], FP16)
        wz_t = persist.tile([P, ND, S], FP16)
        wu_t = persist.tile([P, ND, EH], FP16)
        wo_t = persist.tile([P, ND, D], FP16)
        gq_t = persist.tile([S, T], FP16)
        bq_t = persist.tile([S, T], FP16)
        if with_bu:
            bu_t = persist.tile([P, ND], F32)
            nc.gpsimd.dma_start(
                out=bu_t, in_=bu_d.rearrange("(c p) -> p c", p=P))
            rstd_b = persist.tile([P, T], F32)
        if with_bv:
            bvb = persist.tile([P, EH], FP16)
            nc.gpsimd.dma_start(out=bvb, in_=bass.AP(
                tensor=bv_d.tensor, offset=bv_d.offset, ap=[[0, P], [1, EH]]))
            rstd_kcol = persist.tile([P, 32], F32)
            rkb = persist.tile([P, TK], F32)

        # shared streaming pools (top level: used in prologue and main loop)
        mp = ctx.enter_context(tc.tile_pool(name="mainp", bufs=1))
        sw = ctx.enter_context(tc.tile_pool(name="statp", bufs=1))

        def stats_chain(xt, j, i, mvs):
            """DVE bn_stats: xt[:, j, :] -> mvs[:, i, :] = (mean, var)."""
            st = sw.tile([P, 3, 6], F32, tag="bnst", bufs=2)
            for g in range(3):
                nc.vector.bn_stats(
                    out=st[:, g, :], in_=xt[:, j, g * 256:(g + 1) * 256])
            nc.vector.bn_aggr(out=mvs[:, i, :], in_=st)

        def batch_rstd(mvs, cols, c15, c05, extra_col=None):
            """(mean,var) cols -> rstd^1.5 / rstd^0.5 columns (batched)."""
            c0, cn = cols
            sd = sw.tile([P, 16], F32, tag="sd", bufs=2)
            nc.scalar.activation(
                out=sd[:, :cn], in_=mvs[:, c0:c0 + cn, 1], func=AF.Sqrt,
                bias=eps_t, scale=1.0)
            rst = sw.tile([P, 16], F32, tag="rst", bufs=2)
            nc.vector.reciprocal(out=rst[:, :cn], in_=sd[:, :cn])
            nc.scalar.activation(
                out=c05[:, c0:c0 + cn], in_=rst[:, :cn], func=AF.Sqrt)
            nc.vector.tensor_mul(
                out=c15[:, c0:c0 + cn], in0=rst[:, :cn],
                in1=c05[:, c0:c0 + cn])
            if extra_col is not None:
                nc.vector.tensor_mul(
                    out=extra_col[:, c0:c0 + cn], in0=c05[:, c0:c0 + cn],
                    in1=c05[:, c0:c0 + cn])

        def row_bounce(col, scr, rows, b_out, b_off, b_len, eng=None):
            """col [128, 32] -> DVE 32x32 block transpose -> DRAM rows in
            token order -> stride-0 partition-broadcast read into b_out."""
            eng = eng or nc.gpsimd
            r0, rn = rows
            tr = sw.tile([P, 32], F32, tag="tr", bufs=2)
            nc.vector.transpose(out=tr, in_=col)
            for pb in range(4):
                eng.dma_start(
                    out=scr[r0:r0 + rn, 32 * pb:32 * pb + 32],
                    in_=tr[32 * pb + r0:32 * pb + r0 + rn, :])
            eng.dma_start(out=b_out[:, b_off:b_off + b_len], in_=bass.AP(
                tensor=scr.tensor, offset=scr.offset + r0 * P,
                ap=[[0, P], [1, b_len]]))

        def full_seg_rows(s, eng=None):
            """rstd^1.5/^0.5 rows + bq2 for full-T segment s."""
            batch_rstd(mvs_f, (4 * s, 4), r15_col, r05_col)
            row_bounce(r15_col, s15_d, (4 * s, 4), r15b, s * TT, TT, eng=eng)
            row_bounce(r05_col, s05_d, (4 * s, 4), r05b, s * TT, TT, eng=eng)
            sl = slice(s * TT, (s + 1) * TT)
            if with_bu:
                # general path: q affine carries plain rstd; u evac scales
                nc.vector.tensor_mul(
                    out=rstd_b[:, sl], in0=r05b[:, sl], in1=r05b[:, sl])
                nc.vector.tensor_copy(out=bq2[:, sl], in_=bq_t[:, sl])
            else:
                nc.vector.tensor_mul(
                    out=bq2[:, sl], in0=bq_t[:, sl], in1=r05b[:S, sl])

        # ---------- prologue: stats + k/v on compacted keys ----------
        with (
            tc.tile_pool(name="pw", bufs=1) as pw,
            tc.tile_pool(name="pp", bufs=1, space="PSUM") as pp,
        ):
            # scalar queue: stat streams only (own DMA ring; split into
            # small pieces so the bn chains start early and pipeline with
            # the remaining transfers without starving the xkT loads)
            xks = pw.tile([P, NK, D], FP16)
            for (c0, cn) in _segs(NK, 3):
                nc.scalar.dma_start(
                    out=xks[:, c0:c0 + cn, :], in_=xk_r2[:, c0:c0 + cn, :])
            xf0 = pw.tile([P, 4, D], FP16)
            nc.scalar.dma_start(out=xf0, in_=x_r2[:, 0:4, :])
            # weight/param loads. gpsimd queue: needed-first order.
            nc.gpsimd.dma_start(out=wz_t, in_=wz_r)
            wv_t = pw.tile([P, ND, EH], FP16)
            for c in range(ND):
                nc.gpsimd.dma_start(out=wv_t[:, c, :], in_=wv_r[:, c, :])
            gk_t = pw.tile([S, TK], FP16)
            nc.gpsimd.dma_start(out=gk_t, in_=gk_d)
            bk_t = pw.tile([S, TK], FP16)
            nc.gpsimd.dma_start(out=bk_t, in_=bk_d)
            for c in range(ND):
                nc.gpsimd.dma_start(out=wu_t[:, c, :], in_=wu_r[:, c, :])
            nc.gpsimd.dma_start(out=wo_t, in_=wo_r)
            # sync queue: x for the z/v matmuls + fused stat streams
            xkT_t = pw.tile([P, ND, TK], FP16)
            for c in range(ND):
                nc.sync.dma_start(out=xkT_t[:, c, :], in_=xkT_r[:, c, :])
            xb0 = mp.tile([P, ND, TT], FP16, tag="xb", bufs=2)
            nc.sync.dma_start(out=xb0, in_=xT_r[:, :, 0:TT])

            # PE: z on compacted keys (held in PSUM until the k affine).
            zsegs = _segs(TK, TT)
            assert len(zsegs) <= 3, "zk PSUM ring supports NK <= 12"
            zk_ps = []
            for (s0, sl) in zsegs:
                zp = pp.tile([S, TT], F32, tag="zk", bufs=3)
                for c in range(ND):
                    nc.tensor.matmul(
                        zp[:, :sl], wz_t[:, c, :], xkT_t[:, c, s0:s0 + sl],
                        start=(c == 0), stop=(c == ND - 1))
                zk_ps.append((zp, s0, sl))

            # PE: v on compacted keys; kept/seg0 stats interleaved.
            vgrp = [(ch, e0, ew) for ch in range(NK)
                    for (e0, ew) in ((0, 384), (384, 384))]
            n_inter = len(vgrp)

            def pro_stats(i):
                if i < NK:
                    stats_chain(xks, i, i, mvs_k)
                if i < 4:
                    stats_chain(xf0, i, i, mvs_f)
                if i == NK:
                    nc.gpsimd.dma_start(out=gq_t, in_=gq_d)
                    nc.gpsimd.dma_start(out=bq_t, in_=bq_d)
                if i == NK + 1:
                    batch_rstd(mvs_k, (0, NK), r15k_col, r05k_col,
                               extra_col=rstd_kcol if with_bv else None)
                    row_bounce(r15k_col, s15k_d, (0, NK), r15kb, 0, TK,
                               eng=nc.sync)
                    row_bounce(r05k_col, s05k_d, (0, NK), r05kb, 0, TK,
                               eng=nc.sync)
                    if with_bv:
                        nc.vector.tensor_mul(
                            out=rkb, in0=r05kb, in1=r05kb)
                if i == NK + 2:
                    full_seg_rows(0, eng=nc.sync)

            if n_inter < NK + 3:       # tiny-NK fallback: emit upfront
                for i in range(NK + 3):
                    pro_stats(i)
            for i, (ch, e0, ew) in enumerate(vgrp):
                if n_inter >= NK + 3:
                    pro_stats(i)
                vp = pp.tile([P, 384], F32, tag="vm", bufs=3)
                for c in range(ND):
                    nc.tensor.matmul(
                        vp, xkT_t[:, c, ch * P:(ch + 1) * P],
                        wv_t[:, c, e0:e0 + ew],
                        start=(c == 0), stop=(c == ND - 1))
                if with_bv:
                    nc.scalar.activation(
                        out=v_t[:, ch, e0:e0 + ew], in_=vp, func=AF.Copy,
                        scale=rstd_kcol[:, ch:ch + 1])
                    nc.vector.tensor_add(
                        out=v_t[:, ch, e0:e0 + ew],
                        in0=v_t[:, ch, e0:e0 + ew], in1=bvb[:, e0:e0 + ew])
                else:
                    nc.scalar.copy(out=v_t[:, ch, e0:e0 + ew], in_=vp)

            # k affine. fast path: k'' = zk*(r15kb*gamma) + beta*r05kb
            # (the extra sqrt(rstd_k) cancels against v's missing LN scale
            # through relu^2 homogeneity). general (bv): true affine.
            kgain = rkb if with_bv else r15kb
            if with_bv:
                nc.vector.tensor_copy(out=bk2, in_=bk_t)
            else:
                nc.vector.tensor_mul(out=bk2, in0=bk_t, in1=r05kb[:S, :])
            for (zp, s0, sl) in zk_ps:
                t1 = sw.tile([S, TT], FP16, tag="kt1", bufs=2)
                nc.vector.tensor_mul(
                    out=t1[:, :sl], in0=zp[:, :sl], in1=kgain[:S, s0:s0 + sl])
                t2 = sw.tile([S, TT], FP16, tag="kt2", bufs=2)
                nc.vector.tensor_mul(
                    out=t2[:, :sl], in0=t1[:, :sl], in1=gk_t[:, s0:s0 + sl])
                nc.vector.tensor_add(
                    out=kT[:, s0:s0 + sl], in0=t2[:, :sl],
                    in1=bk2[:, s0:s0 + sl])

            # q(tb0) matmuls at prologue end (affine drains into main loop)
            zq0 = pp.tile([S, TT], F32, tag="q0", bufs=1)
            for c in range(ND):
                nc.tensor.matmul(
                    zq0, wz_t[:, c, :], xb0[:, c, :],
                    start=(c == 0), stop=(c == ND - 1))
            qgain = rstd_b if with_bu else r15b
            t1 = sw.tile([S, TT], FP16, tag="qt1", bufs=2)
            nc.vector.tensor_mul(out=t1, in0=zq0, in1=qgain[:S, 0:TT])
            t2 = sw.tile([S, TT], FP16, tag="qt2", bufs=2)
            nc.vector.tensor_mul(out=t2, in0=t1, in1=gq_t[:, 0:TT])
            qT_cur = mp.tile([S, TT], FP16, tag="qT", bufs=2)
            nc.vector.tensor_add(out=qT_cur, in0=t2, in1=bq2[:, 0:TT])

        # ---------- fused main loop over 512-query blocks ----------
        # Full-T stat chains 4..15 + row segs 1..3 run inside the loop:
        # seg s is consumed by the q affine of block s, emitted in
        # iteration s-1, so chains 4s..4s+3 are emitted just before it.
        xfs_cur = [None]

        def late_stats(tb, phase):
            if tb > 2:
                return
            c0 = 4 * (tb + 1)
            if phase == 0:
                xfs = sw.tile([P, 4, D], FP16, tag="xfs", bufs=2)
                nc.sync.dma_start(out=xfs, in_=x_r2[:, c0:c0 + 4, :])
                xfs_cur[0] = xfs
                for j in (0, 1):
                    stats_chain(xfs, j, c0 + j, mvs_f)
            else:
                for j in (2, 3):
                    stats_chain(xfs_cur[0], j, c0 + j, mvs_f)
                full_seg_rows(tb + 1)

        with (
            tc.tile_pool(name="mw", bufs=1) as mw,
            tc.tile_pool(name="ps", bufs=1, space="PSUM") as ps,
        ):
            xb_cur = xb0
            for tb in range(NTT):
                ts_ = slice(tb * TT, (tb + 1) * TT)
                if tb < NTT - 1:
                    xb_next = mp.tile([P, ND, TT], FP16, tag="xb", bufs=2)
                    nc.sync.dma_start(
                        out=xb_next,
                        in_=xT_r[:, :, (tb + 1) * TT:(tb + 2) * TT])

                # --- qk + a = relu(qk)^2 (relu alternates DVE/ACT) ---
                aTt = mw.tile([P, NK, TT], FP16, tag="aT", bufs=2)
                for uc in range(NK):
                    qkp = ps.tile([P, TT], F32, tag="qk", bufs=3)
                    nc.tensor.matmul(
                        qkp, kT[:, uc * P:(uc + 1) * P], qT_cur,
                        start=True, stop=True)
                    rt = mw.tile([P, TT], FP16, tag="rt", bufs=3)
                    if uc % 3 == 2:
                        nc.scalar.activation(out=rt, in_=qkp, func=AF.Relu)
                    else:
                        nc.vector.tensor_scalar_max(rt, qkp, 0.0)
                    nc.vector.tensor_mul(out=aTt[:, uc, :], in0=rt, in1=rt)
                late_stats(tb, 0)

                # --- u (plain evac: rstd_q rides in via the q affine) ---
                uTt = mw.tile([P, ND, TT], FP16, tag="uT", bufs=2)
                for e in range(ND):
                    up = ps.tile([P, TT], F32, tag="mm", bufs=3)
                    for c in range(ND):
                        nc.tensor.matmul(
                            up, wu_t[:, c, e * P:(e + 1) * P], xb_cur[:, c, :],
                            start=(c == 0), stop=(c == ND - 1))
                    if with_bu:
                        uf = mw.tile([P, TT], F32, tag="uf", bufs=2)
                        nc.vector.tensor_mul(
                            out=uf, in0=up, in1=rstd_b[:, ts_])
                        nc.scalar.activation(
                            out=uTt[:, e, :], in_=uf, func=AF.Identity,
                            bias=bu_t[:, e:e + 1])
                    else:
                        nc.scalar.copy(out=uTt[:, e, :], in_=up)
                late_stats(tb, 1)

                # --- attn = v^T @ a^T, then g = u * attn in place ---
                for e in range(ND):
                    ap_ = ps.tile([P, TT], F32, tag="at", bufs=2)
                    for uc in range(NK):
                        nc.tensor.matmul(
                            ap_, v_t[:, uc, e * P:(e + 1) * P], aTt[:, uc, :],
                            start=(uc == 0), stop=(uc == NK - 1))
                    nc.vector.tensor_mul(
                        out=uTt[:, e, :], in0=ap_, in1=uTt[:, e, :])

                # --- q for the next block ---
                if tb < NTT - 1:
                    nts = slice((tb + 1) * TT, (tb + 2) * TT)
                    zp = ps.tile([S, TT], F32, tag="mm", bufs=3)
                    for c in range(ND):
                        nc.tensor.matmul(
                            zp, wz_t[:, c, :], xb_next[:, c, :],
                            start=(c == 0), stop=(c == ND - 1))
                    qgain = rstd_b if with_bu else r15b
                    t1 = mw.tile([S, TT], FP16, tag="t1", bufs=2)
                    nc.vector.tensor_mul(out=t1, in0=zp, in1=qgain[:S, nts])
                    t2 = mw.tile([S, TT], FP16, tag="t2", bufs=2)
                    nc.vector.tensor_mul(out=t2, in0=t1, in1=gq_t[:, nts])
                    qT_next = mp.tile([S, TT], FP16, tag="qT", bufs=2)
                    nc.vector.tensor_add(
                        out=qT_next, in0=t2, in1=bq2[:, nts])

                # --- out = g^T @ Wo (plain copy; rstd_q rides in u) ---
                for tch in range(TT // P):
                    it = tb * (TT // P) + tch
                    tc_ = slice(it * P, (it + 1) * P)
                    osb = mw.tile([P, D], FP16, tag="osb", bufs=3)
                    for (d0, dw) in ((0, 384), (384, 384)):
                        op_ = ps.tile([P, TT], F32, tag="mm", bufs=3)
                        for e in range(ND):
                            nc.tensor.matmul(
                                op_[:, :dw], uTt[:, e, tch * P:(tch + 1) * P],
                                wo_t[:, e, d0:d0 + dw],
                                start=(e == 0), stop=(e == ND - 1))
                        nc.scalar.copy(out=osb[:, d0:d0 + dw], in_=op_[:, :dw])
                    nc.sync.dma_start(out=out_d[tc_, :], in_=osb)

                if tb < NTT - 1:
                    xb_cur = xb_next
                    qT_cur = qT_next

    nc.finalize()
    return nc


def prep_core_inputs(inputs):
    """Host-side prep: fold LN centering/scale into weights, fold bz into
    the q/k affines, gather kept (unmasked) key tokens, slice E halves."""
    f = np.float32
    x = np.asarray(inputs["x"], f)
    mask = np.asarray(inputs["mask"])
    ln_w = np.asarray(inputs["ln_w"], f)
    ln_b = np.asarray(inputs["ln_b"], f)
    Wz = np.asarray(inputs["Wz"], f)
    bz = np.asarray(inputs["bz"], f)
    Wu = np.asarray(inputs["Wu"], f)
    bu = np.asarray(inputs["bu"], f)
    Wv = np.asarray(inputs["Wv"], f)
    bv = np.asarray(inputs["bv"], f)
    Wo = np.asarray(inputs["Wo"], f)
    gq = np.asarray(inputs["gamma_q"], f)
    bq = np.asarray(inputs["beta_q"], f)
    gk = np.asarray(inputs["gamma_k"], f)
    bk = np.asarray(inputs["beta_k"], f)

    # fold ln_w into weights, then fold the LN centering projection:
    # ((x - mu) * rstd) @ W = rstd * (x @ (W - colmean(W)))
    Wz_e = ln_w[:, None] * Wz
    Wu_e = ln_w[:, None] * Wu
    Wv_e = ln_w[:, None] * Wv
    Wz_c = (Wz_e - Wz_e.mean(0, keepdims=True)).astype(NPFP16)
    Wu_c = (Wu_e - Wu_e.mean(0, keepdims=True)).astype(NPFP16)
    Wv_c = (Wv_e - Wv_e.mean(0, keepdims=True)).astype(NPFP16)
    # biases: z-path bias folds exactly into the affines
    bz_e = ln_b @ Wz + bz
    bu_e = ln_b @ Wu + bu
    bv_e = ln_b @ Wv + bv
    with_bu = bool(np.any(bu_e != 0))
    with_bv = bool(np.any(bv_e != 0))

    bq_f = bq + bz_e[None, :] * gq      # [T, S]
    bk_f = bk + bz_e[None, :] * gk

    keeps = [np.where(~mask[b])[0] for b in range(B)]
    NK = max(1, -(-max(len(kk) for kk in keeps) // P))
    TK = NK * P

    gqT = np.ascontiguousarray(gq.T.astype(NPFP16))
    bqT = np.ascontiguousarray(bq_f.T.astype(NPFP16))

    in_maps = []
    for c in range(N_CORES):
        b, h = c // 2, c % 2
        cols = slice(h * EH, (h + 1) * EH)
        kidx = keeps[b]
        nk = len(kidx)
        xk = np.zeros((TK, D), NPFP16)
        xk[:nk] = x[b][kidx].astype(NPFP16)
        gkT = np.zeros((S, TK), NPFP16)
        gkT[:, :nk] = gk[kidx].T.astype(NPFP16)
        bkT = np.zeros((S, TK), NPFP16)
        bkT[:, :nk] = bk_f[kidx].T.astype(NPFP16)
        m = {
            "x": np.ascontiguousarray(x[b].astype(NPFP16)),
            "xT": np.ascontiguousarray(x[b].T.astype(NPFP16)),
            "xk": xk,
            "xkT": np.ascontiguousarray(xk.T),
            "gqT": gqT,
            "bqT": bqT,
            "gkT": gkT,
            "bkT": bkT,
            "Wz": np.ascontiguousarray(Wz_c),
            "Wu": np.ascontiguousarray(Wu_c[:, cols]),
            "Wv": np.ascontiguousarray(Wv_c[:, cols]),
            "Wo": np.ascontiguousarray(Wo[cols, :].astype(NPFP16)),
        }
        if with_bu:
            m["bu"] = np.ascontiguousarray(bu_e[cols])
        if with_bv:
            m["bv"] = np.ascontiguousarray(
                bv_e[cols].reshape(1, EH).astype(NPFP16))
        in_maps.append(m)
    return in_maps, NK, with_bu, with_bv


def combine_outputs(inputs, parts):
    """parts: list of 8 [T, D] fp16 partials -> full [B, T, D] fp32."""
    f = np.float32
    x = np.asarray(inputs["x"], f)
    bo = np.asarray(inputs["bo"], f)
    out = np.empty((B, T, D), f)
    for b in range(B):
        out[b] = (parts[2 * b].astype(f) + parts[2 * b + 1].astype(f)
                  + bo[None, :] + x[b])
    return out


_NC_CACHE = {}


def run(inputs, trace=False, **kw):
    in_maps, NK, with_bu, with_bv = prep_core_inputs(inputs)
    key = (NK, with_bu, with_bv)
    if key not in _NC_CACHE:
        _NC_CACHE[key] = build_module(NK, with_bu, with_bv)
    nc = _NC_CACHE[key]
    res = run_bass_kernel_spmd(
        nc, in_maps, core_ids=list(range(N_CORES)), trace=trace, **kw
    )
    parts = [r["outp"] for r in res.results]
    return combine_outputs(inputs, parts), res


def kernel(**inputs):
    out, _ = run(inputs)
    return out
